# revision 11
# baseline (speedup 1.0000x reference)
"""Trainium2 Bass kernel for nn_CrossModalAttention (B=2, LQ=LK=2048,
QDIM=HID=1024, KDIM=VDIM=768, H=16, D=64).

Sharding: 8 cores = 2 batches x 4 head-groups (4 heads each).
Per core: q/k/v projections column-sliced over HID, attention for its 4
heads, row-parallel partial of the out-projection. Host sums the 4
partials per batch (the row-parallel unshard) and adds bo.

Device dataflow (per core), all matmuls in bf16 (~4.5e-3 rel):
  - host passes query/key/value[b] transposed (and K/V key-compacted:
    query_mask masks the KEY axis globally per batch, so masked keys are
    dropped on host and the remainder padded to a multiple of 128)
  - qT/kT [hid, tokens] and v [keys, hid] computed on device
  - attention runs as a single software-pipelined stream over units
    (p, tb): scores for unit i+1 are emitted while PV matmuls for unit
    i drain, so the PE never stalls on the ACT-engine exp; qproj /
    outproj chunks are woven into the stream as thunks
  - per head pair (row-packed K=64 matmuls via tile_position):
    scoresT [keys, q] -> ACT exp(s/8 + mask_bias) -> PV matmul with a
    ones-augmented V (M=65) giving ctxT and the softmax denominator
  - normalize on DVE (reciprocal + gpsimd partition-broadcast)
  - out-projection from ctxT, partial written to DRAM per tb
"""

import math

import ml_dtypes
import numpy as np

B, LQ, LK = 2, 2048, 2048
QDIM, KDIM, VDIM, HID, H = 1024, 768, 768, 1024, 16
D = HID // H  # 64
HG = 4  # head-groups (cores per batch)
HL = H // HG  # heads per core = 4
GH = HL * D  # per-core hid slice = 256
N_CORES = 8
TB = 512  # token block
NTB = LQ // TB  # 4
NEG = -1.0e30

BF16 = True
PROFILE = False
LAST_EXEC_NS = None
LAST_TRACE_DIR = None

_CACHE = {}


def _build(nkt: int, with_bv: bool, bf16: bool):
    import concourse.bacc as bacc
    import concourse.mybir as mybir
    import concourse.tile as tile

    nkeys = nkt * 128
    # key blocks of <=512 for the k-projection
    kbs = [min(512, nkeys - s) for s in range(0, nkeys, 512)]

    f32 = mybir.dt.float32
    f32r = mybir.dt.bfloat16 if bf16 else mybir.dt.float32r
    f8 = mybir.dt.float8e4
    DR = mybir.MatmulPerfMode.DoubleRow
    Exp = mybir.ActivationFunctionType.Exp
    # host scales xq by 2^4 and wq by 2^8 into fp8e4 (k path stays
    # bf16 for accuracy); the exp's scale folds the 2^12 back out
    EXP_SCALE = 0.125 / float(1 << 12)

    PR_BUFS = nkt + 2  # probs live set: draining unit + scoring unit

    nc = bacc.Bacc(
        "TRN2", target_bir_lowering=False, debug=False, num_devices=N_CORES
    )

    # DRAM tensors (per-core shapes)
    XQ = nc.dram_tensor("xq", [128, 8, LQ], f8, kind="ExternalInput").ap()
    XK = nc.dram_tensor("xk", [128, 6, nkeys], f32r, kind="ExternalInput").ap()
    XV = nc.dram_tensor("xv", [128, 6, nkeys], f32r, kind="ExternalInput").ap()
    WQ = nc.dram_tensor("wq", [128, 8, GH], f8, kind="ExternalInput").ap()
    WK = nc.dram_tensor("wk", [128, 6, GH], f32r, kind="ExternalInput").ap()
    WV = nc.dram_tensor("wv", [128, 6, GH], f32r, kind="ExternalInput").ap()
    WO = nc.dram_tensor("wo", [128, 2, QDIM], f32r, kind="ExternalInput").ap()
    MB = nc.dram_tensor("mbias", [128, nkt], f32, kind="ExternalInput").ap()
    BQ = nc.dram_tensor("bqk", [128, 4], f32, kind="ExternalInput").ap()
    BV = None
    if with_bv:
        BV = nc.dram_tensor("bv", [128, 2], f32, kind="ExternalInput").ap()
    OUT = nc.dram_tensor("outp", [LQ, QDIM], f32r, kind="ExternalOutput").ap()

    with tile.TileContext(nc) as tc:
        with (
            tc.tile_pool(name="consts", bufs=1) as consts,
            tc.tile_pool(name="resid", bufs=1) as resid,
            tc.tile_pool(name="xs", bufs=2) as xs,
            tc.tile_pool(name="probs", bufs=4) as probs_pool,
            tc.tile_pool(name="norm", bufs=3) as norm_pool,
            tc.tile_pool(name="outs", bufs=3) as outs_pool,
            tc.tile_pool(name="ps", bufs=2, space="PSUM") as ps,
        ):
            # ---- constants / weights ----
            # wk/wq gate the critical path: put them on the scalar HWDGE
            # queue (idle at start) instead of the slow gpsimd SWDGE ring;
            # wv/wo are needed later and stay on gpsimd so the sync ring
            # keeps streaming inputs
            warm = consts.tile([1, 512], f32r)
            nc.vector.memset(warm, 1.0)
            wq_sb = consts.tile([128, 8, GH], f8)
            wk_sb = consts.tile([128, 6, GH], f32r)
            wv_sb = consts.tile([128, 6, GH], f32r)
            wo_sb = consts.tile([128, 2, QDIM], f32r)
            mb_sb = consts.tile([128, nkt], f32)
            bqk_sb = consts.tile([128, 4], f32)
            nc.scalar.dma_start(out=bqk_sb, in_=BQ)
            nc.scalar.dma_start(out=mb_sb, in_=MB)
            nc.gpsimd.dma_start(out=wv_sb, in_=WV)
            nc.gpsimd.dma_start(out=wo_sb, in_=WO)
            bv_sb = None
            if with_bv:
                bv_sb = consts.tile([128, 2], f32)
                nc.gpsimd.dma_start(out=bv_sb, in_=BV)

            # ---- PE warm-up ----
            # the tensor engine p-state ramps to full clock only after ~3us
            # of continuous work; burn trivial matmuls during the startup
            # DMA wait so the real kproj runs at full speed
            for _ in range(8):
                wps = ps.tile([128, 512], f32, tag="proj", name="warm_ps")
                nc.tensor.matmul(
                    wps[0:1, :], warm[0:1, 0:1], warm, start=True, stop=True
                )

            # ---- residents ----
            # qT tiles double as ctxT tiles later (WAR handled by Tile)
            qT = [resid.tile([128, LQ], f32r, tag=f"qT{p}", name=f"qT{p}") for p in range(2)]
            kT = [resid.tile([128, nkeys], f32r, tag=f"kT{p}", name=f"kT{p}") for p in range(2)]
            v_sb = resid.tile([128, nkt, HL, D + 1], f32r)
            # ones columns for the denominator rows: fill the whole tile,
            # the v-projection copies then overwrite the [., ., ., 0:D] part
            if bf16:
                nc.vector.memset(v_sb, 1.0)
            else:
                nc.vector.memset(v_sb[:, :, :, :].bitcast(f32), 1.0)

            # ---- k projection (per key-block) ----
            def emit_xk(kb_i):
                kbw = kbs[kb_i]
                s0 = kb_i * 512
                xk_t = xs.tile([128, 6, 512], f32r, tag="xk", name="xk_t", bufs=3)
                nc.sync.dma_start(
                    out=xk_t[:, :, :kbw], in_=XK[:, :, s0 : s0 + kbw]
                )
                return xk_t

            def emit_kproj_m(kb_i, m, xk_t):
                kbw = kbs[kb_i]
                s0 = kb_i * 512
                ps_t = ps.tile([128, 512], f32, tag="proj", name="kp_ps")
                for k in range(6):
                    nc.tensor.matmul(
                        ps_t[:, :kbw],
                        wk_sb[:, k, m * 128 : (m + 1) * 128],
                        xk_t[:, k, :kbw],
                        start=(k == 0),
                        stop=(k == 5),
                    )
                nc.vector.tensor_scalar_add(
                    kT[m][:, s0 : s0 + kbw],
                    ps_t[:, :kbw],
                    bqk_sb[:, 2 + m : 3 + m],
                )

            # ---- v projection ----
            vproj_state = {}

            def emit_vproj_kt(kt):
                kb_i = kt // 4
                sub = kt % 4
                if kb_i not in vproj_state:
                    kbw = kbs[kb_i]
                    xv_t = xs.tile(
                        [128, 6, 512], f32r, tag="xv", name="xv_t"
                    )
                    nc.sync.dma_start(
                        out=xv_t[:, :, :kbw],
                        in_=XV[:, :, kb_i * 512 : kb_i * 512 + kbw],
                    )
                    vproj_state[kb_i] = xv_t
                xv_t = vproj_state[kb_i]
                ps_t = ps.tile([128, 512], f32, tag="proj", name="vp_ps")
                for k in range(6):
                    nc.tensor.matmul(
                        ps_t[:, :GH],
                        xv_t[:, k, sub * 128 : (sub + 1) * 128],
                        wv_sb[:, k, :],
                        start=(k == 0),
                        stop=(k == 5),
                    )
                nc.vector.tensor_copy(
                    v_sb[:, kt, :, 0:D],
                    ps_t[:, :GH].rearrange("p (h d) -> p h d", h=HL),
                )

            def emit_xq(tb):
                t0 = tb * TB
                xq_t = xs.tile([128, 8, TB], f8, tag="xq", name="xq_t")
                nc.sync.dma_start(out=xq_t, in_=XQ[:, :, t0 : t0 + TB])
                return xq_t

            def emit_qproj_m(tb, m, xq_t):
                t0 = tb * TB
                ps_t = ps.tile([128, 512], f32, tag="proj", name="qp_ps")
                for j in range(4):
                    nc.tensor.matmul(
                        ps_t,
                        wq_sb[:, 2 * j : 2 * j + 2, m * 128 : (m + 1) * 128],
                        xq_t[:, 2 * j : 2 * j + 2, :],
                        start=(j == 0),
                        stop=(j == 3),
                        perf_mode=DR,
                    )
                nc.vector.tensor_scalar_add(
                    qT[m][:, t0 : t0 + TB],
                    ps_t,
                    bqk_sb[:, m : m + 1],
                )

            def emit_scores(p, tb, kt):
                t0 = tb * TB
                k0 = kt * 128
                sc = ps.tile([128, 2, TB], f32, tag="sc", name="sc")
                for hh in range(2):
                    nc.tensor.matmul(
                        sc[:, hh, :],
                        kT[p][hh * 64 : hh * 64 + 64, k0 : k0 + 128],
                        qT[p][hh * 64 : hh * 64 + 64, t0 : t0 + TB],
                        start=True,
                        stop=True,
                        tile_position=(hh * 64, 0),
                    )
                pr = probs_pool.tile(
                    [128, 2, TB], f32r, tag="pr", name="pr", bufs=PR_BUFS
                )
                nc.scalar.activation(
                    pr, sc, Exp, bias=mb_sb[:, kt : kt + 1], scale=EXP_SCALE
                )
                return pr

            def emit_pv(p, tb, kt, pr, ctx_ps):
                for hh in range(2):
                    nc.tensor.matmul(
                        ctx_ps[hh],
                        v_sb[:, kt, 2 * p + hh, :],
                        pr[:, hh, :],
                        start=(kt == 0),
                        stop=(kt == nkt - 1),
                    )

            def emit_normalize(p, tb, ctx_ps, direct=False):
                t0 = tb * TB
                if direct:
                    # tail variant: nobody is waiting on these PSUM banks,
                    # so skip the big ctx evacuation copies; only the 1-row
                    # denominator is copied out (custom-DVE reciprocal can't
                    # read PSUM)
                    for hh in range(2):
                        dcp = norm_pool.tile([1, TB], f32, tag="dcp", name="dcp")
                        nc.vector.tensor_copy(dcp, ctx_ps[hh][D : D + 1, :])
                        rec1 = norm_pool.tile([1, TB], f32, tag="rec1", name="rec1")
                        nc.vector.reciprocal_approx_fast(out=rec1, in_=dcp)
                        rbc = norm_pool.tile([D, TB], f32, tag="rbc", name="rbc")
                        nc.gpsimd.partition_broadcast(rbc, rec1)
                        dst = qT[p][hh * 64 : hh * 64 + 64, t0 : t0 + TB]
                        nc.vector.tensor_mul(dst, ctx_ps[hh][0:D, :], rbc)
                        if with_bv:
                            nc.vector.tensor_scalar_add(
                                dst, dst, bv_sb[64 * hh : 64 * hh + 64, p : p + 1]
                            )
                    return
                # evacuate both PSUM ctx tiles to SBUF first (quick DVE
                # copies) so the next unit's PV matmuls get their PSUM
                # slots immediately; the normalize chain then runs from
                # SBUF off the PE critical path
                evac = []
                for hh in range(2):
                    ctmp = norm_pool.tile([D, TB], f32, tag="ctmp", name="ctmp")
                    nc.vector.tensor_copy(ctmp, ctx_ps[hh][0:D, :])
                    dcp = norm_pool.tile([1, TB], f32, tag="dcp", name="dcp")
                    nc.vector.tensor_copy(dcp, ctx_ps[hh][D : D + 1, :])
                    evac.append((ctmp, dcp))
                for hh in range(2):
                    ctmp, dcp = evac[hh]
                    rbc = norm_pool.tile([D, TB], f32, tag="rbc", name="rbc")
                    nc.gpsimd.partition_broadcast(rbc, dcp)
                    rec = norm_pool.tile([D, TB], f32, tag="rec", name="rec")
                    nc.vector.reciprocal_approx_fast(out=rec, in_=rbc)
                    dst = qT[p][hh * 64 : hh * 64 + 64, t0 : t0 + TB]
                    nc.vector.tensor_mul(dst, ctmp, rec)
                    if with_bv:
                        nc.vector.tensor_scalar_add(
                            dst, dst, bv_sb[64 * hh : 64 * hh + 64, p : p + 1]
                        )

            def emit_outproj_nh(tt, nh, on_scalar=False):
                ps_t = ps.tile([128, 512], f32, tag="proj", name="op_ps")
                for kk in range(2):
                    nc.tensor.matmul(
                        ps_t,
                        qT[kk][:, tt * 128 : (tt + 1) * 128],
                        wo_sb[:, kk, nh * 512 : (nh + 1) * 512],
                        start=(kk == 0),
                        stop=(kk == 1),
                    )
                o_sb = outs_pool.tile([128, 512], f32r, tag="osb", name="o_sb")
                # in the tail the ACT engine is idle; use it for the evac
                # so the PE<->DVE ping-pong disappears
                if on_scalar:
                    nc.scalar.copy(o_sb, ps_t)
                else:
                    nc.vector.tensor_copy(o_sb, ps_t)
                nc.sync.dma_start(
                    out=OUT[
                        tt * 128 : (tt + 1) * 128, nh * 512 : (nh + 1) * 512
                    ],
                    in_=o_sb,
                )

            def qproj_m_thunks(tb, m, cell):
                # 4 (m=0, incl. the xq prefetch) or 3 (m=1) small thunks;
                # chunks chain into one psum accumulation group
                def start(tb=tb):
                    cell["xq"] = emit_xq(tb)

                def chunk(j0, j1, fin, tb=tb, m=m):
                    if j0 == 0:
                        cell[m] = ps.tile(
                            [128, 512], f32, tag="proj", name="qp_ps"
                        )
                    ps_t = cell[m]
                    for j in range(j0, j1):
                        nc.tensor.matmul(
                            ps_t,
                            wq_sb[:, 2 * j : 2 * j + 2, m * 128 : (m + 1) * 128],
                            cell["xq"][:, 2 * j : 2 * j + 2, :],
                            start=(j == 0),
                            stop=(j == 3),
                            perf_mode=DR,
                        )
                    if fin:
                        nc.vector.tensor_scalar_add(
                            qT[m][:, tb * TB : (tb + 1) * TB],
                            ps_t,
                            bqk_sb[:, m : m + 1],
                        )

                out = [] if m else [start]
                out += [
                    lambda: chunk(0, 2, False),
                    lambda: chunk(2, 4, True),
                ]
                return out

            def outproj_thunks(tb, half, on_scalar=False):
                out = []
                for tt in range(4 * tb + 2 * half, 4 * tb + 2 * half + 2):
                    for nh in range(2):
                        out.append(
                            lambda tt=tt, nh=nh: emit_outproj_nh(
                                tt, nh, on_scalar
                            )
                        )
                return out

            # ---- emission schedule ----
            # prologue: kproj/qproj m0, prefill unit (0,0) scores so the
            # ACT engine starts exp-ing early, then the m1 halves + vproj
            xk0 = xs.tile([128, 6, 512], f32r, tag="xk", name="xk_t", bufs=3)
            for k in range(6):
                nc.sync.dma_start(
                    out=wk_sb[:, k, :], in_=WK[:, k, :]
                )
                nc.sync.dma_start(
                    out=xk0[:, k, : kbs[0]], in_=XK[:, k, : kbs[0]]
                )
            xk_ts = [xk0]
            nc.sync.dma_start(out=wq_sb, in_=WQ)
            xq0 = emit_xq(0)
            xk_ts += [emit_xk(kb_i) for kb_i in range(1, len(kbs))]
            emit_kproj_m(0, 0, xk_ts[0])
            emit_qproj_m(0, 0, xq0)
            n_pre = min(4, nkt)
            prs0 = [emit_scores(0, 0, kt) for kt in range(n_pre)]
            for kb_i in range(1, len(kbs)):
                emit_kproj_m(kb_i, 0, xk_ts[kb_i])
            # remaining prefill scores run at ACT pace (the 2-buf sc
            # rotation waits on exp); weave the m1-half projections in
            # between so the PE keeps busy
            fill = [
                (lambda kb_i=kb_i: emit_kproj_m(kb_i, 1, xk_ts[kb_i]))
                for kb_i in range(len(kbs))
            ] + [lambda: emit_qproj_m(0, 1, xq0)]
            fi = 0
            for kt in range(n_pre, nkt):
                prs0.append(emit_scores(0, 0, kt))
                if fi < len(fill):
                    fill[fi]()
                    fi += 1
            while fi < len(fill):
                fill[fi]()
                fi += 1
            n_vpre = min(4, nkt)
            for kt in range(n_vpre):
                emit_vproj_kt(kt)

            # unit-lag pipeline: while unit (p,tb) emits scores+exp, the
            # previous unit's PV matmuls drain against its ready probs.
            # vproj kt>=4 rides as unit-1 thunks: thunk[i] runs at step i,
            # and pv(0,0,kt) (needing v_sb[kt]) runs at step kt > i.
            units = [(p, tb) for tb in range(NTB) for p in range(2)]
            qcells = {tb: {} for tb in range(1, NTB)}
            vp_thunks = [
                (lambda kt=kt: emit_vproj_kt(kt)) for kt in range(n_vpre, nkt)
            ]
            thunk_map = {
                1: (lambda qp: [
                    th
                    for i in range(max(len(vp_thunks), len(qp)))
                    for th in (
                        vp_thunks[i : i + 1] + qp[i : i + 1]
                    )
                ])(qproj_m_thunks(1, 0, qcells[1])),
                2: qproj_m_thunks(1, 1, qcells[1]),
                3: qproj_m_thunks(2, 0, qcells[2]) + outproj_thunks(0, 0),
                4: qproj_m_thunks(2, 1, qcells[2]) + outproj_thunks(0, 1),
                5: qproj_m_thunks(3, 0, qcells[3]) + outproj_thunks(1, 0),
                6: qproj_m_thunks(3, 1, qcells[3]) + outproj_thunks(1, 1),
                7: outproj_thunks(2, 0) + outproj_thunks(2, 1),
            }
            prev_prs = prs0
            for ui in range(1, len(units)):
                p, tb = units[ui]
                pp, ptb = units[ui - 1]
                ctx_prev = [
                    ps.tile([D + 1, TB], f32, tag="ctx", name=f"ctx{pp}_{ptb}_{i}")
                    for i in range(2)
                ]
                thunks = thunk_map.get(ui, [])
                ti = 0
                cur_prs = []
                for kt in range(nkt):
                    cur_prs.append(emit_scores(p, tb, kt))
                    emit_pv(pp, ptb, kt, prev_prs[kt], ctx_prev)
                    if ti < len(thunks):
                        thunks[ti]()
                        ti += 1
                while ti < len(thunks):
                    thunks[ti]()
                    ti += 1
                emit_normalize(pp, ptb, ctx_prev)
                prev_prs = cur_prs
            # drain the last unit (1, NTB-1)
            ctx_last = [
                ps.tile([D + 1, TB], f32, tag="ctx", name=f"ctx_last_{i}")
                for i in range(2)
            ]
            for kt in range(nkt):
                emit_pv(1, NTB - 1, kt, prev_prs[kt], ctx_last)
            emit_normalize(1, NTB - 1, ctx_last, direct=True)
            for half in range(2):
                for th in outproj_thunks(3, half, on_scalar=True):
                    th()

    nc.compile()
    return nc


def kernel(
    query, key, value, Wq, bq, Wk, bk, Wv, bv, Wo, bo, query_mask, key_mask
):
    global LAST_EXEC_NS, LAST_TRACE_DIR
    from concourse.bass_utils import run_bass_kernel_spmd

    query = np.asarray(query, dtype=np.float32)
    key = np.asarray(key, dtype=np.float32)
    value = np.asarray(value, dtype=np.float32)
    Wq = np.asarray(Wq, dtype=np.float32)
    Wk = np.asarray(Wk, dtype=np.float32)
    Wv = np.asarray(Wv, dtype=np.float32)
    Wo = np.asarray(Wo, dtype=np.float32)
    bq = np.asarray(bq, dtype=np.float32)
    bk = np.asarray(bk, dtype=np.float32)
    bv = np.asarray(bv, dtype=np.float32)
    bo = np.asarray(bo, dtype=np.float32)
    qm = np.asarray(query_mask)
    km = np.asarray(key_mask)

    # host-side key compaction (query_mask masks the KEY axis, globally
    # per batch)
    keep = [np.flatnonzero(qm[b] != 0) for b in range(B)]
    nkeep = max((len(k) for k in keep), default=0)
    nkt = max(1, math.ceil(nkeep / 128))
    nkeys = nkt * 128

    with_bv = bool(np.any(bv))
    ck = (nkt, with_bv, BF16)
    if ck not in _CACHE:
        _CACHE[ck] = _build(nkt, with_bv, BF16)
    nc = _CACHE[ck]

    wdt = ml_dtypes.bfloat16 if BF16 else np.float32
    f8dt = ml_dtypes.float8_e4m3fn
    SX, SW = 16.0, 256.0  # fp8 pre-scales for the q/k path (2^4, 2^8)

    def arr_kmajor(a, ktiles, dt=None, s=1.0):  # [dim, n] -> [128, ktiles, n]
        return np.ascontiguousarray(
            (a * s if s != 1.0 else a)
            .reshape(ktiles, 128, a.shape[1])
            .transpose(1, 0, 2)
        ).astype(dt or wdt)

    in_maps = []
    for c in range(N_CORES):
        b, hg = c // HG, c % HG
        hs = hg * GH
        idx = keep[b]
        # compacted + padded key/value (transposed)
        xk = np.zeros((KDIM, nkeys), np.float32)
        xk[:, : len(idx)] = key[b].T[:, idx]
        xv = np.zeros((VDIM, nkeys), np.float32)
        xv[:, : len(idx)] = value[b].T[:, idx]
        mbias = np.full((nkeys,), NEG, np.float32)
        mbias[: len(idx)] = 0.0
        # qT/kT live on-device scaled by SX*SW = 2^12
        bqk = np.empty((128, 4), np.float32)
        bqk[:, 0] = bq[hs : hs + 128] * (SX * SW)
        bqk[:, 1] = bq[hs + 128 : hs + 256] * (SX * SW)
        bqk[:, 2] = bk[hs : hs + 128]
        bqk[:, 3] = bk[hs + 128 : hs + 256]
        m = {
            "xq": arr_kmajor(query[b].T, 8, f8dt, SX),
            "xk": arr_kmajor(xk, 6),
            "xv": arr_kmajor(xv, 6),
            "wq": arr_kmajor(Wq[:, hs : hs + GH], 8, f8dt, SW),
            "wk": arr_kmajor(Wk[:, hs : hs + GH], 6),
            "wv": arr_kmajor(Wv[:, hs : hs + GH], 6),
            "wo": arr_kmajor(Wo[hs : hs + GH, :], 2),
            "mbias": np.ascontiguousarray(mbias.reshape(nkt, 128).T),
            "bqk": bqk,
        }
        if with_bv:
            bvt = np.empty((128, 2), np.float32)
            bvt[:, 0] = bv[hs : hs + 128]
            bvt[:, 1] = bv[hs + 128 : hs + 256]
            m["bv"] = bvt
        in_maps.append(m)

    kwargs = {}
    if PROFILE:
        import tempfile

        LAST_TRACE_DIR = tempfile.mkdtemp(prefix="bass_trace_")
        kwargs = {"trace": True, "tmpdir": LAST_TRACE_DIR}
    res = run_bass_kernel_spmd(nc, in_maps, list(range(N_CORES)), **kwargs)
    LAST_EXEC_NS = res.exec_time_ns

    out = np.zeros((B, LQ, QDIM), np.float32)
    for c in range(N_CORES):
        out[c // HG] += res.results[c]["outp"].astype(np.float32)
    out += bo[None, None, :]
    for b in range(B):
        if len(keep[b]) == 0:
            # all keys masked: reference softmax is NaN everywhere
            out[b] = np.nan
    # key_mask masks the QUERY axis in the reference; a zero row makes the
    # whole softmax row -inf -> NaN output for that query position.
    for b in range(B):
        zq = np.flatnonzero(km[b] == 0)
        if len(zq):
            out[b, zq, :] = np.nan
    return out


# revision 12
# speedup vs baseline: 1.0279x; 1.0279x over previous
"""Trainium2 Bass kernel for nn_CrossModalAttention (B=2, LQ=LK=2048,
QDIM=HID=1024, KDIM=VDIM=768, H=16, D=64).

Sharding: 8 cores = 2 batches x 4 head-groups (4 heads each).
Per core: q/k/v projections column-sliced over HID, attention for its 4
heads, row-parallel partial of the out-projection. Host sums the 4
partials per batch (the row-parallel unshard) and adds bo.

Device dataflow (per core), all matmuls in bf16 (~4.5e-3 rel):
  - host passes query/key/value[b] transposed (and K/V key-compacted:
    query_mask masks the KEY axis globally per batch, so masked keys are
    dropped on host and the remainder padded to a multiple of 128)
  - qT/kT [hid, tokens] and v [keys, hid] computed on device
  - attention runs as a single software-pipelined stream over units
    (p, tb): scores for unit i+1 are emitted while PV matmuls for unit
    i drain, so the PE never stalls on the ACT-engine exp; qproj /
    outproj chunks are woven into the stream as thunks
  - per head pair (row-packed K=64 matmuls via tile_position):
    scoresT [keys, q] -> ACT exp(s/8 + mask_bias) -> PV matmul with a
    ones-augmented V (M=65) giving ctxT and the softmax denominator
  - normalize on DVE (reciprocal + gpsimd partition-broadcast)
  - out-projection from ctxT, partial written to DRAM per tb
"""

import math

import ml_dtypes
import numpy as np

B, LQ, LK = 2, 2048, 2048
QDIM, KDIM, VDIM, HID, H = 1024, 768, 768, 1024, 16
D = HID // H  # 64
HG = 4  # head-groups (cores per batch)
HL = H // HG  # heads per core = 4
GH = HL * D  # per-core hid slice = 256
N_CORES = 8
TB = 512  # token block
NTB = LQ // TB  # 4
NEG = -1.0e30

BF16 = True
PROFILE = False
LAST_EXEC_NS = None
LAST_TRACE_DIR = None

_CACHE = {}


def _build(nkt: int, with_bv: bool, bf16: bool):
    import concourse.bacc as bacc
    import concourse.mybir as mybir
    import concourse.tile as tile

    nkeys = nkt * 128
    # key blocks of <=512 for the k-projection
    kbs = [min(512, nkeys - s) for s in range(0, nkeys, 512)]

    f32 = mybir.dt.float32
    f32r = mybir.dt.bfloat16 if bf16 else mybir.dt.float32r
    f8 = mybir.dt.float8e4
    DR = mybir.MatmulPerfMode.DoubleRow
    Exp = mybir.ActivationFunctionType.Exp
    # host scales xq by 2^4 and wq by 2^8 into fp8e4 (k path stays
    # bf16 for accuracy); the exp's scale folds the 2^12 back out
    EXP_SCALE = 0.125 / float(1 << 12)

    PR_BUFS = nkt + 2  # probs live set: draining unit + scoring unit

    nc = bacc.Bacc(
        "TRN2", target_bir_lowering=False, debug=False, num_devices=N_CORES
    )

    # DRAM tensors (per-core shapes)
    XQ = nc.dram_tensor("xq", [128, 8, LQ], f8, kind="ExternalInput").ap()
    XK = nc.dram_tensor("xk", [128, 6, nkeys], f32r, kind="ExternalInput").ap()
    XV = nc.dram_tensor("xv", [128, 6, nkeys], f32r, kind="ExternalInput").ap()
    WQ = nc.dram_tensor("wq", [128, 8, GH], f8, kind="ExternalInput").ap()
    WK = nc.dram_tensor("wk", [128, 6, GH], f32r, kind="ExternalInput").ap()
    WV = nc.dram_tensor("wv", [128, 6, GH], f32r, kind="ExternalInput").ap()
    WO = nc.dram_tensor("wo", [128, 2, QDIM], f32r, kind="ExternalInput").ap()
    MB = nc.dram_tensor("mbias", [128, nkt], f32, kind="ExternalInput").ap()
    BQ = nc.dram_tensor("bqk", [128, 4], f32, kind="ExternalInput").ap()
    BV = None
    if with_bv:
        BV = nc.dram_tensor("bv", [128, 2], f32, kind="ExternalInput").ap()
    OUT = nc.dram_tensor("outp", [LQ, QDIM], f32r, kind="ExternalOutput").ap()

    with tile.TileContext(nc) as tc:
        with (
            tc.tile_pool(name="consts", bufs=1) as consts,
            tc.tile_pool(name="resid", bufs=1) as resid,
            tc.tile_pool(name="xs", bufs=2) as xs,
            tc.tile_pool(name="probs", bufs=4) as probs_pool,
            tc.tile_pool(name="norm", bufs=3) as norm_pool,
            tc.tile_pool(name="outs", bufs=3) as outs_pool,
            tc.tile_pool(name="ps", bufs=2, space="PSUM") as ps,
        ):
            # ---- constants / weights ----
            # wk/wq gate the critical path: put them on the scalar HWDGE
            # queue (idle at start) instead of the slow gpsimd SWDGE ring;
            # wv/wo are needed later and stay on gpsimd so the sync ring
            # keeps streaming inputs
            warm = consts.tile([1, 512], f32r)
            nc.vector.memset(warm, 1.0)
            wq_sb = consts.tile([128, 8, GH], f8)
            wk_sb = consts.tile([128, 6, GH], f32r)
            wv_sb = consts.tile([128, 6, GH], f32r)
            wo_sb = consts.tile([128, 2, QDIM], f32r)
            mb_sb = consts.tile([128, nkt], f32)
            bqk_sb = consts.tile([128, 4], f32)
            nc.scalar.dma_start(out=bqk_sb, in_=BQ)
            nc.scalar.dma_start(out=mb_sb, in_=MB)
            nc.gpsimd.dma_start(out=wv_sb, in_=WV)
            nc.gpsimd.dma_start(out=wo_sb, in_=WO)
            bv_sb = None
            if with_bv:
                bv_sb = consts.tile([128, 2], f32)
                nc.gpsimd.dma_start(out=bv_sb, in_=BV)

            # ---- PE warm-up ----
            # the tensor engine p-state ramps to full clock only after ~3us
            # of continuous work; burn trivial matmuls during the startup
            # DMA wait so the real kproj runs at full speed
            for _ in range(8):
                wps = ps.tile([128, 512], f32, tag="proj", name="warm_ps")
                nc.tensor.matmul(
                    wps[0:1, :], warm[0:1, 0:1], warm, start=True, stop=True
                )

            # ---- residents ----
            # qT tiles double as ctxT tiles later (WAR handled by Tile)
            qT = [resid.tile([128, LQ], f32r, tag=f"qT{p}", name=f"qT{p}") for p in range(2)]
            kT = [resid.tile([128, nkeys], f32r, tag=f"kT{p}", name=f"kT{p}") for p in range(2)]
            v_sb = resid.tile([128, nkt, HL, D + 1], f32r)
            # ones columns for the denominator rows: fill the whole tile,
            # the v-projection copies then overwrite the [., ., ., 0:D] part
            if bf16:
                nc.vector.memset(v_sb, 1.0)
            else:
                nc.vector.memset(v_sb[:, :, :, :].bitcast(f32), 1.0)

            # ---- k projection (per key-block) ----
            def emit_xk(kb_i):
                kbw = kbs[kb_i]
                s0 = kb_i * 512
                xk_t = xs.tile([128, 6, 512], f32r, tag="xk", name="xk_t", bufs=3)
                nc.sync.dma_start(
                    out=xk_t[:, :, :kbw], in_=XK[:, :, s0 : s0 + kbw]
                )
                return xk_t

            def emit_kproj_m(kb_i, m, xk_t):
                kbw = kbs[kb_i]
                s0 = kb_i * 512
                ps_t = ps.tile([128, 512], f32, tag="proj", name="kp_ps")
                for k in range(6):
                    nc.tensor.matmul(
                        ps_t[:, :kbw],
                        wk_sb[:, k, m * 128 : (m + 1) * 128],
                        xk_t[:, k, :kbw],
                        start=(k == 0),
                        stop=(k == 5),
                    )
                nc.vector.tensor_scalar_add(
                    kT[m][:, s0 : s0 + kbw],
                    ps_t[:, :kbw],
                    bqk_sb[:, 2 + m : 3 + m],
                )

            # ---- v projection ----
            vproj_state = {}

            def emit_vproj_kt(kt):
                kb_i = kt // 4
                sub = kt % 4
                if kb_i not in vproj_state:
                    kbw = kbs[kb_i]
                    xv_t = xs.tile(
                        [128, 6, 512], f32r, tag="xv", name="xv_t"
                    )
                    nc.sync.dma_start(
                        out=xv_t[:, :, :kbw],
                        in_=XV[:, :, kb_i * 512 : kb_i * 512 + kbw],
                    )
                    vproj_state[kb_i] = xv_t
                xv_t = vproj_state[kb_i]
                ps_t = ps.tile([128, 512], f32, tag="proj", name="vp_ps")
                for k in range(6):
                    nc.tensor.matmul(
                        ps_t[:, :GH],
                        xv_t[:, k, sub * 128 : (sub + 1) * 128],
                        wv_sb[:, k, :],
                        start=(k == 0),
                        stop=(k == 5),
                    )
                nc.vector.tensor_copy(
                    v_sb[:, kt, :, 0:D],
                    ps_t[:, :GH].rearrange("p (h d) -> p h d", h=HL),
                )

            def emit_xq(tb):
                t0 = tb * TB
                xq_t = xs.tile([128, 8, TB], f8, tag="xq", name="xq_t")
                nc.sync.dma_start(out=xq_t, in_=XQ[:, :, t0 : t0 + TB])
                return xq_t

            def emit_qproj_m(tb, m, xq_t):
                t0 = tb * TB
                ps_t = ps.tile([128, 512], f32, tag="proj", name="qp_ps")
                for j in range(4):
                    nc.tensor.matmul(
                        ps_t,
                        wq_sb[:, 2 * j : 2 * j + 2, m * 128 : (m + 1) * 128],
                        xq_t[:, 2 * j : 2 * j + 2, :],
                        start=(j == 0),
                        stop=(j == 3),
                        perf_mode=DR,
                    )
                nc.vector.tensor_scalar_add(
                    qT[m][:, t0 : t0 + TB],
                    ps_t,
                    bqk_sb[:, m : m + 1],
                )

            def emit_scores(p, tb, kt):
                t0 = tb * TB
                k0 = kt * 128
                sc = ps.tile([128, 2, TB], f32, tag="sc", name="sc")
                for hh in range(2):
                    nc.tensor.matmul(
                        sc[:, hh, :],
                        kT[p][hh * 64 : hh * 64 + 64, k0 : k0 + 128],
                        qT[p][hh * 64 : hh * 64 + 64, t0 : t0 + TB],
                        start=True,
                        stop=True,
                        tile_position=(hh * 64, 0),
                    )
                pr = probs_pool.tile(
                    [128, 2, TB], f32r, tag="pr", name="pr", bufs=PR_BUFS
                )
                nc.scalar.activation(
                    pr, sc, Exp, bias=mb_sb[:, kt : kt + 1], scale=EXP_SCALE
                )
                return pr

            def emit_pv(p, tb, kt, pr, ctx_ps):
                for hh in range(2):
                    nc.tensor.matmul(
                        ctx_ps[hh],
                        v_sb[:, kt, 2 * p + hh, :],
                        pr[:, hh, :],
                        start=(kt == 0),
                        stop=(kt == nkt - 1),
                    )

            def emit_normalize(p, tb, ctx_ps, direct=False):
                t0 = tb * TB
                if direct:
                    # tail variant: nobody is waiting on these PSUM banks,
                    # so skip the big ctx evacuation copies; only the 1-row
                    # denominator is copied out (custom-DVE reciprocal can't
                    # read PSUM)
                    for hh in range(2):
                        dcp = norm_pool.tile([1, TB], f32, tag="dcp", name="dcp")
                        nc.vector.tensor_copy(dcp, ctx_ps[hh][D : D + 1, :])
                        rec1 = norm_pool.tile([1, TB], f32, tag="rec1", name="rec1")
                        nc.vector.reciprocal_approx_fast(out=rec1, in_=dcp)
                        rbc = norm_pool.tile([D, TB], f32, tag="rbc", name="rbc")
                        nc.gpsimd.partition_broadcast(rbc, rec1)
                        dst = qT[p][hh * 64 : hh * 64 + 64, t0 : t0 + TB]
                        nc.vector.tensor_mul(dst, ctx_ps[hh][0:D, :], rbc)
                        if with_bv:
                            nc.vector.tensor_scalar_add(
                                dst, dst, bv_sb[64 * hh : 64 * hh + 64, p : p + 1]
                            )
                    return
                # evacuate both PSUM ctx tiles to SBUF first (quick DVE
                # copies) so the next unit's PV matmuls get their PSUM
                # slots immediately; the normalize chain then runs from
                # SBUF off the PE critical path
                evac = []
                for hh in range(2):
                    ctmp = norm_pool.tile([D, TB], f32, tag="ctmp", name="ctmp")
                    nc.vector.tensor_copy(ctmp, ctx_ps[hh][0:D, :])
                    dcp = norm_pool.tile([1, TB], f32, tag="dcp", name="dcp")
                    nc.vector.tensor_copy(dcp, ctx_ps[hh][D : D + 1, :])
                    evac.append((ctmp, dcp))
                for hh in range(2):
                    ctmp, dcp = evac[hh]
                    rbc = norm_pool.tile([D, TB], f32, tag="rbc", name="rbc")
                    nc.gpsimd.partition_broadcast(rbc, dcp)
                    rec = norm_pool.tile([D, TB], f32, tag="rec", name="rec")
                    nc.vector.reciprocal_approx_fast(out=rec, in_=rbc)
                    dst = qT[p][hh * 64 : hh * 64 + 64, t0 : t0 + TB]
                    nc.vector.tensor_mul(dst, ctmp, rec)
                    if with_bv:
                        nc.vector.tensor_scalar_add(
                            dst, dst, bv_sb[64 * hh : 64 * hh + 64, p : p + 1]
                        )

            def emit_outproj_nh(tt, nh, on_scalar=False):
                ps_t = ps.tile([128, 512], f32, tag="proj", name="op_ps")
                for kk in range(2):
                    nc.tensor.matmul(
                        ps_t,
                        qT[kk][:, tt * 128 : (tt + 1) * 128],
                        wo_sb[:, kk, nh * 512 : (nh + 1) * 512],
                        start=(kk == 0),
                        stop=(kk == 1),
                    )
                o_sb = outs_pool.tile([128, 512], f32r, tag="osb", name="o_sb")
                # in the tail the ACT engine is idle; use it for the evac
                # so the PE<->DVE ping-pong disappears
                if on_scalar:
                    nc.scalar.copy(o_sb, ps_t)
                else:
                    nc.vector.tensor_copy(o_sb, ps_t)
                nc.sync.dma_start(
                    out=OUT[
                        tt * 128 : (tt + 1) * 128, nh * 512 : (nh + 1) * 512
                    ],
                    in_=o_sb,
                )

            def qproj_m_thunks(tb, m, cell):
                # 4 (m=0, incl. the xq prefetch) or 3 (m=1) small thunks;
                # chunks chain into one psum accumulation group
                def start(tb=tb):
                    cell["xq"] = emit_xq(tb)

                def chunk(j0, j1, fin, tb=tb, m=m):
                    if j0 == 0:
                        cell[m] = ps.tile(
                            [128, 512], f32, tag="proj", name="qp_ps"
                        )
                    ps_t = cell[m]
                    for j in range(j0, j1):
                        nc.tensor.matmul(
                            ps_t,
                            wq_sb[:, 2 * j : 2 * j + 2, m * 128 : (m + 1) * 128],
                            cell["xq"][:, 2 * j : 2 * j + 2, :],
                            start=(j == 0),
                            stop=(j == 3),
                            perf_mode=DR,
                        )
                    if fin:
                        nc.vector.tensor_scalar_add(
                            qT[m][:, tb * TB : (tb + 1) * TB],
                            ps_t,
                            bqk_sb[:, m : m + 1],
                        )

                out = [] if m else [start]
                out += [
                    lambda: chunk(0, 2, False),
                    lambda: chunk(2, 4, True),
                ]
                return out

            def outproj_thunks(tb, half, on_scalar=False):
                out = []
                for tt in range(4 * tb + 2 * half, 4 * tb + 2 * half + 2):
                    for nh in range(2):
                        out.append(
                            lambda tt=tt, nh=nh: emit_outproj_nh(
                                tt, nh, on_scalar
                            )
                        )
                return out

            # ---- emission schedule ----
            # prologue: kproj/qproj m0, prefill unit (0,0) scores so the
            # ACT engine starts exp-ing early, then the m1 halves + vproj
            xk_ts = [emit_xk(0)]
            nc.sync.dma_start(out=wk_sb, in_=WK)
            nc.sync.dma_start(out=wq_sb, in_=WQ)
            xq0 = emit_xq(0)
            xk_ts += [emit_xk(kb_i) for kb_i in range(1, len(kbs))]
            emit_kproj_m(0, 0, xk_ts[0])
            emit_qproj_m(0, 0, xq0)
            n_pre = min(4, nkt)
            prs0 = [emit_scores(0, 0, kt) for kt in range(n_pre)]
            for kb_i in range(1, len(kbs)):
                emit_kproj_m(kb_i, 0, xk_ts[kb_i])
            # remaining prefill scores run at ACT pace (the 2-buf sc
            # rotation waits on exp); weave the m1-half projections in
            # between so the PE keeps busy
            fill = [
                (lambda kb_i=kb_i: emit_kproj_m(kb_i, 1, xk_ts[kb_i]))
                for kb_i in range(len(kbs))
            ] + [lambda: emit_qproj_m(0, 1, xq0)]
            fi = 0
            for kt in range(n_pre, nkt):
                prs0.append(emit_scores(0, 0, kt))
                if fi < len(fill):
                    fill[fi]()
                    fi += 1
            while fi < len(fill):
                fill[fi]()
                fi += 1
            n_vpre = min(4, nkt)
            for kt in range(n_vpre):
                emit_vproj_kt(kt)

            # unit-lag pipeline: while unit (p,tb) emits scores+exp, the
            # previous unit's PV matmuls drain against its ready probs.
            # vproj kt>=4 rides as unit-1 thunks: thunk[i] runs at step i,
            # and pv(0,0,kt) (needing v_sb[kt]) runs at step kt > i.
            units = [(p, tb) for tb in range(NTB) for p in range(2)]
            qcells = {tb: {} for tb in range(1, NTB)}
            vp_thunks = [
                (lambda kt=kt: emit_vproj_kt(kt)) for kt in range(n_vpre, nkt)
            ]
            thunk_map = {
                1: (lambda qp: [
                    th
                    for i in range(max(len(vp_thunks), len(qp)))
                    for th in (
                        vp_thunks[i : i + 1] + qp[i : i + 1]
                    )
                ])(qproj_m_thunks(1, 0, qcells[1])),
                2: qproj_m_thunks(1, 1, qcells[1]),
                3: qproj_m_thunks(2, 0, qcells[2]) + outproj_thunks(0, 0),
                4: qproj_m_thunks(2, 1, qcells[2]) + outproj_thunks(0, 1),
                5: qproj_m_thunks(3, 0, qcells[3]) + outproj_thunks(1, 0),
                6: qproj_m_thunks(3, 1, qcells[3]) + outproj_thunks(1, 1),
                7: outproj_thunks(2, 0) + outproj_thunks(2, 1),
            }
            prev_prs = prs0
            for ui in range(1, len(units)):
                p, tb = units[ui]
                pp, ptb = units[ui - 1]
                ctx_prev = [
                    ps.tile([D + 1, TB], f32, tag="ctx", name=f"ctx{pp}_{ptb}_{i}")
                    for i in range(2)
                ]
                thunks = thunk_map.get(ui, [])
                ti = 0
                cur_prs = []
                for kt in range(nkt):
                    cur_prs.append(emit_scores(p, tb, kt))
                    emit_pv(pp, ptb, kt, prev_prs[kt], ctx_prev)
                    if ti < len(thunks):
                        thunks[ti]()
                        ti += 1
                while ti < len(thunks):
                    thunks[ti]()
                    ti += 1
                emit_normalize(pp, ptb, ctx_prev)
                prev_prs = cur_prs
            # drain the last unit (1, NTB-1)
            ctx_last = [
                ps.tile([D + 1, TB], f32, tag="ctx", name=f"ctx_last_{i}")
                for i in range(2)
            ]
            for kt in range(nkt):
                emit_pv(1, NTB - 1, kt, prev_prs[kt], ctx_last)
            emit_normalize(1, NTB - 1, ctx_last, direct=True)
            for half in range(2):
                for th in outproj_thunks(3, half, on_scalar=True):
                    th()

    nc.compile()
    return nc


def kernel(
    query, key, value, Wq, bq, Wk, bk, Wv, bv, Wo, bo, query_mask, key_mask
):
    global LAST_EXEC_NS, LAST_TRACE_DIR
    from concourse.bass_utils import run_bass_kernel_spmd

    query = np.asarray(query, dtype=np.float32)
    key = np.asarray(key, dtype=np.float32)
    value = np.asarray(value, dtype=np.float32)
    Wq = np.asarray(Wq, dtype=np.float32)
    Wk = np.asarray(Wk, dtype=np.float32)
    Wv = np.asarray(Wv, dtype=np.float32)
    Wo = np.asarray(Wo, dtype=np.float32)
    bq = np.asarray(bq, dtype=np.float32)
    bk = np.asarray(bk, dtype=np.float32)
    bv = np.asarray(bv, dtype=np.float32)
    bo = np.asarray(bo, dtype=np.float32)
    qm = np.asarray(query_mask)
    km = np.asarray(key_mask)

    # host-side key compaction (query_mask masks the KEY axis, globally
    # per batch)
    keep = [np.flatnonzero(qm[b] != 0) for b in range(B)]
    nkeep = max((len(k) for k in keep), default=0)
    nkt = max(1, math.ceil(nkeep / 128))
    nkeys = nkt * 128

    with_bv = bool(np.any(bv))
    ck = (nkt, with_bv, BF16)
    if ck not in _CACHE:
        _CACHE[ck] = _build(nkt, with_bv, BF16)
    nc = _CACHE[ck]

    wdt = ml_dtypes.bfloat16 if BF16 else np.float32
    f8dt = ml_dtypes.float8_e4m3fn
    SX, SW = 16.0, 256.0  # fp8 pre-scales for the q/k path (2^4, 2^8)

    def arr_kmajor(a, ktiles, dt=None, s=1.0):  # [dim, n] -> [128, ktiles, n]
        return np.ascontiguousarray(
            (a * s if s != 1.0 else a)
            .reshape(ktiles, 128, a.shape[1])
            .transpose(1, 0, 2)
        ).astype(dt or wdt)

    in_maps = []
    for c in range(N_CORES):
        b, hg = c // HG, c % HG
        hs = hg * GH
        idx = keep[b]
        # compacted + padded key/value (transposed)
        xk = np.zeros((KDIM, nkeys), np.float32)
        xk[:, : len(idx)] = key[b].T[:, idx]
        xv = np.zeros((VDIM, nkeys), np.float32)
        xv[:, : len(idx)] = value[b].T[:, idx]
        mbias = np.full((nkeys,), NEG, np.float32)
        mbias[: len(idx)] = 0.0
        # qT/kT live on-device scaled by SX*SW = 2^12
        bqk = np.empty((128, 4), np.float32)
        bqk[:, 0] = bq[hs : hs + 128] * (SX * SW)
        bqk[:, 1] = bq[hs + 128 : hs + 256] * (SX * SW)
        bqk[:, 2] = bk[hs : hs + 128]
        bqk[:, 3] = bk[hs + 128 : hs + 256]
        m = {
            "xq": arr_kmajor(query[b].T, 8, f8dt, SX),
            "xk": arr_kmajor(xk, 6),
            "xv": arr_kmajor(xv, 6),
            "wq": arr_kmajor(Wq[:, hs : hs + GH], 8, f8dt, SW),
            "wk": arr_kmajor(Wk[:, hs : hs + GH], 6),
            "wv": arr_kmajor(Wv[:, hs : hs + GH], 6),
            "wo": arr_kmajor(Wo[hs : hs + GH, :], 2),
            "mbias": np.ascontiguousarray(mbias.reshape(nkt, 128).T),
            "bqk": bqk,
        }
        if with_bv:
            bvt = np.empty((128, 2), np.float32)
            bvt[:, 0] = bv[hs : hs + 128]
            bvt[:, 1] = bv[hs + 128 : hs + 256]
            m["bv"] = bvt
        in_maps.append(m)

    kwargs = {}
    if PROFILE:
        import tempfile

        LAST_TRACE_DIR = tempfile.mkdtemp(prefix="bass_trace_")
        kwargs = {"trace": True, "tmpdir": LAST_TRACE_DIR}
    res = run_bass_kernel_spmd(nc, in_maps, list(range(N_CORES)), **kwargs)
    LAST_EXEC_NS = res.exec_time_ns

    out = np.zeros((B, LQ, QDIM), np.float32)
    for c in range(N_CORES):
        out[c // HG] += res.results[c]["outp"].astype(np.float32)
    out += bo[None, None, :]
    for b in range(B):
        if len(keep[b]) == 0:
            # all keys masked: reference softmax is NaN everywhere
            out[b] = np.nan
    # key_mask masks the QUERY axis in the reference; a zero row makes the
    # whole softmax row -inf -> NaN output for that query position.
    for b in range(B):
        zq = np.flatnonzero(km[b] == 0)
        if len(zq):
            out[b, zq, :] = np.nan
    return out


# revision 14
# speedup vs baseline: 1.0331x; 1.0051x over previous
"""Trainium2 Bass kernel for nn_CrossModalAttention (B=2, LQ=LK=2048,
QDIM=HID=1024, KDIM=VDIM=768, H=16, D=64).

Sharding: 8 cores = 2 batches x 4 head-groups (4 heads each).
Per core: q/k/v projections column-sliced over HID, attention for its 4
heads, row-parallel partial of the out-projection. Host sums the 4
partials per batch (the row-parallel unshard) and adds bo.

Device dataflow (per core), all matmuls in bf16 (~4.5e-3 rel):
  - host passes query/key/value[b] transposed (and K/V key-compacted:
    query_mask masks the KEY axis globally per batch, so masked keys are
    dropped on host and the remainder padded to a multiple of 128)
  - qT/kT [hid, tokens] and v [keys, hid] computed on device
  - attention runs as a single software-pipelined stream over units
    (p, tb): scores for unit i+1 are emitted while PV matmuls for unit
    i drain, so the PE never stalls on the ACT-engine exp; qproj /
    outproj chunks are woven into the stream as thunks
  - per head pair (row-packed K=64 matmuls via tile_position):
    scoresT [keys, q] -> ACT exp(s/8 + mask_bias) -> PV matmul with a
    ones-augmented V (M=65) giving ctxT and the softmax denominator
  - normalize on DVE (reciprocal + gpsimd partition-broadcast)
  - out-projection from ctxT, partial written to DRAM per tb
"""

import math

import ml_dtypes
import numpy as np

B, LQ, LK = 2, 2048, 2048
QDIM, KDIM, VDIM, HID, H = 1024, 768, 768, 1024, 16
D = HID // H  # 64
HG = 4  # head-groups (cores per batch)
HL = H // HG  # heads per core = 4
GH = HL * D  # per-core hid slice = 256
N_CORES = 8
TB = 512  # token block
NTB = LQ // TB  # 4
NEG = -1.0e30

BF16 = True
PROFILE = False
LAST_EXEC_NS = None
LAST_TRACE_DIR = None

_CACHE = {}


def _build(nkt: int, with_bv: bool, bf16: bool):
    import concourse.bacc as bacc
    import concourse.mybir as mybir
    import concourse.tile as tile

    nkeys = nkt * 128
    # key blocks of <=512 for the k-projection
    kbs = [min(512, nkeys - s) for s in range(0, nkeys, 512)]

    f32 = mybir.dt.float32
    f32r = mybir.dt.bfloat16 if bf16 else mybir.dt.float32r
    f8 = mybir.dt.float8e4
    DR = mybir.MatmulPerfMode.DoubleRow
    Exp = mybir.ActivationFunctionType.Exp
    # host scales xq by 2^4 and wq by 2^8 into fp8e4 (k path stays
    # bf16 for accuracy); the exp's scale folds the 2^12 back out
    EXP_SCALE = 0.125 / float(1 << 12)

    PR_BUFS = nkt + 2  # probs live set: draining unit + scoring unit

    nc = bacc.Bacc(
        "TRN2", target_bir_lowering=False, debug=False, num_devices=N_CORES
    )

    # DRAM tensors (per-core shapes)
    XQ = nc.dram_tensor("xq", [128, 8, LQ], f8, kind="ExternalInput").ap()
    XK = nc.dram_tensor("xk", [128, 6, nkeys], f32r, kind="ExternalInput").ap()
    XV = nc.dram_tensor("xv", [128, 6, nkeys], f32r, kind="ExternalInput").ap()
    WQ = nc.dram_tensor("wq", [128, 8, GH], f8, kind="ExternalInput").ap()
    WK = nc.dram_tensor("wk", [128, 6, GH], f32r, kind="ExternalInput").ap()
    WV = nc.dram_tensor("wv", [128, 6, GH], f32r, kind="ExternalInput").ap()
    WO = nc.dram_tensor("wo", [128, 2, QDIM], f32r, kind="ExternalInput").ap()
    MB = nc.dram_tensor("mbias", [128, nkt], f32, kind="ExternalInput").ap()
    BQ = nc.dram_tensor("bqk", [128, 4], f32, kind="ExternalInput").ap()
    BV = None
    if with_bv:
        BV = nc.dram_tensor("bv", [128, 2], f32, kind="ExternalInput").ap()
    OUT = nc.dram_tensor("outp", [LQ, QDIM], f32r, kind="ExternalOutput").ap()

    with tile.TileContext(nc) as tc:
        with (
            tc.tile_pool(name="consts", bufs=1) as consts,
            tc.tile_pool(name="resid", bufs=1) as resid,
            tc.tile_pool(name="xs", bufs=2) as xs,
            tc.tile_pool(name="probs", bufs=4) as probs_pool,
            tc.tile_pool(name="norm", bufs=3) as norm_pool,
            tc.tile_pool(name="outs", bufs=3) as outs_pool,
            tc.tile_pool(name="ps", bufs=2, space="PSUM") as ps,
        ):
            # ---- constants / weights ----
            # wk/wq gate the critical path: put them on the scalar HWDGE
            # queue (idle at start) instead of the slow gpsimd SWDGE ring;
            # wv/wo are needed later and stay on gpsimd so the sync ring
            # keeps streaming inputs
            warm = consts.tile([1, 512], f32r)
            nc.vector.memset(warm, 1.0)
            wq_sb = consts.tile([128, 8, GH], f8)
            wk_sb = consts.tile([128, 6, GH], f32r)
            wv_sb = consts.tile([128, 6, GH], f32r)
            wo_sb = consts.tile([128, 2, QDIM], f32r)
            mb_sb = consts.tile([128, nkt], f32)
            bqk_sb = consts.tile([128, 4], f32)
            nc.scalar.dma_start(out=bqk_sb, in_=BQ)
            nc.scalar.dma_start(out=mb_sb, in_=MB)
            nc.gpsimd.dma_start(out=wv_sb, in_=WV)
            nc.gpsimd.dma_start(out=wo_sb, in_=WO)
            bv_sb = None
            if with_bv:
                bv_sb = consts.tile([128, 2], f32)
                nc.gpsimd.dma_start(out=bv_sb, in_=BV)

            # ---- PE warm-up ----
            # the tensor engine p-state ramps to full clock only after ~3us
            # of continuous work; burn trivial matmuls during the startup
            # DMA wait so the real kproj runs at full speed
            for _ in range(8):
                wps = ps.tile([128, 512], f32, tag="proj", name="warm_ps")
                nc.tensor.matmul(
                    wps[0:1, :], warm[0:1, 0:1], warm, start=True, stop=True
                )

            # ---- residents ----
            # qT tiles double as ctxT tiles later (WAR handled by Tile)
            qT = [resid.tile([128, LQ], f32r, tag=f"qT{p}", name=f"qT{p}") for p in range(2)]
            kT = [resid.tile([128, nkeys], f32r, tag=f"kT{p}", name=f"kT{p}") for p in range(2)]
            v_sb = resid.tile([128, nkt, HL, D + 1], f32r)
            # ones columns for the denominator rows: fill the whole tile,
            # the v-projection copies then overwrite the [., ., ., 0:D] part
            if bf16:
                nc.vector.memset(v_sb, 1.0)
            else:
                nc.vector.memset(v_sb[:, :, :, :].bitcast(f32), 1.0)

            # ---- k projection (per key-block) ----
            def emit_xk(kb_i):
                kbw = kbs[kb_i]
                s0 = kb_i * 512
                xk_t = xs.tile([128, 6, 512], f32r, tag="xk", name="xk_t", bufs=3)
                nc.sync.dma_start(
                    out=xk_t[:, :, :kbw], in_=XK[:, :, s0 : s0 + kbw]
                )
                return xk_t

            def emit_kproj_m(kb_i, m, xk_t):
                kbw = kbs[kb_i]
                s0 = kb_i * 512
                ps_t = ps.tile([128, 512], f32, tag="proj", name="kp_ps")
                for k in range(6):
                    nc.tensor.matmul(
                        ps_t[:, :kbw],
                        wk_sb[:, k, m * 128 : (m + 1) * 128],
                        xk_t[:, k, :kbw],
                        start=(k == 0),
                        stop=(k == 5),
                    )
                nc.vector.tensor_scalar_add(
                    kT[m][:, s0 : s0 + kbw],
                    ps_t[:, :kbw],
                    bqk_sb[:, 2 + m : 3 + m],
                )

            # ---- v projection ----
            vproj_state = {}

            def emit_vproj_kt(kt):
                kb_i = kt // 4
                sub = kt % 4
                if kb_i not in vproj_state:
                    kbw = kbs[kb_i]
                    xv_t = xs.tile(
                        [128, 6, 512], f32r, tag="xv", name="xv_t"
                    )
                    nc.sync.dma_start(
                        out=xv_t[:, :, :kbw],
                        in_=XV[:, :, kb_i * 512 : kb_i * 512 + kbw],
                    )
                    vproj_state[kb_i] = xv_t
                xv_t = vproj_state[kb_i]
                ps_t = ps.tile([128, 512], f32, tag="proj", name="vp_ps")
                for k in range(6):
                    nc.tensor.matmul(
                        ps_t[:, :GH],
                        xv_t[:, k, sub * 128 : (sub + 1) * 128],
                        wv_sb[:, k, :],
                        start=(k == 0),
                        stop=(k == 5),
                    )
                nc.vector.tensor_copy(
                    v_sb[:, kt, :, 0:D],
                    ps_t[:, :GH].rearrange("p (h d) -> p h d", h=HL),
                )

            def emit_xq(tb):
                t0 = tb * TB
                xq_t = xs.tile([128, 8, TB], f8, tag="xq", name="xq_t")
                nc.sync.dma_start(out=xq_t, in_=XQ[:, :, t0 : t0 + TB])
                return xq_t

            def emit_qproj_m(tb, m, xq_t):
                t0 = tb * TB
                ps_t = ps.tile([128, 512], f32, tag="proj", name="qp_ps")
                for j in range(4):
                    nc.tensor.matmul(
                        ps_t,
                        wq_sb[:, 2 * j : 2 * j + 2, m * 128 : (m + 1) * 128],
                        xq_t[:, 2 * j : 2 * j + 2, :],
                        start=(j == 0),
                        stop=(j == 3),
                        perf_mode=DR,
                    )
                nc.vector.tensor_scalar_add(
                    qT[m][:, t0 : t0 + TB],
                    ps_t,
                    bqk_sb[:, m : m + 1],
                )

            def emit_scores(p, tb, kt):
                t0 = tb * TB
                k0 = kt * 128
                sc = ps.tile([128, 2, TB], f32, tag="sc", name="sc")
                for hh in range(2):
                    nc.tensor.matmul(
                        sc[:, hh, :],
                        kT[p][hh * 64 : hh * 64 + 64, k0 : k0 + 128],
                        qT[p][hh * 64 : hh * 64 + 64, t0 : t0 + TB],
                        start=True,
                        stop=True,
                        tile_position=(hh * 64, 0),
                    )
                pr = probs_pool.tile(
                    [128, 2, TB], f32r, tag="pr", name="pr", bufs=PR_BUFS
                )
                nc.scalar.activation(
                    pr, sc, Exp, bias=mb_sb[:, kt : kt + 1], scale=EXP_SCALE
                )
                return pr

            def emit_pv(p, tb, kt, pr, ctx_ps):
                for hh in range(2):
                    nc.tensor.matmul(
                        ctx_ps[hh],
                        v_sb[:, kt, 2 * p + hh, :],
                        pr[:, hh, :],
                        start=(kt == 0),
                        stop=(kt == nkt - 1),
                    )

            def emit_normalize(p, tb, ctx_ps, direct=False):
                t0 = tb * TB
                if direct:
                    # tail variant: nobody is waiting on these PSUM banks,
                    # so skip the big ctx evacuation copies; only the 1-row
                    # denominator is copied out (custom-DVE reciprocal can't
                    # read PSUM)
                    for hh in range(2):
                        dcp = norm_pool.tile([1, TB], f32, tag="dcp", name="dcp")
                        nc.vector.tensor_copy(dcp, ctx_ps[hh][D : D + 1, :])
                        rec1 = norm_pool.tile([1, TB], f32, tag="rec1", name="rec1")
                        nc.vector.reciprocal_approx_fast(out=rec1, in_=dcp)
                        rbc = norm_pool.tile([D, TB], f32, tag="rbc", name="rbc")
                        nc.gpsimd.partition_broadcast(rbc, rec1)
                        dst = qT[p][hh * 64 : hh * 64 + 64, t0 : t0 + TB]
                        nc.vector.tensor_mul(dst, ctx_ps[hh][0:D, :], rbc)
                        if with_bv:
                            nc.vector.tensor_scalar_add(
                                dst, dst, bv_sb[64 * hh : 64 * hh + 64, p : p + 1]
                            )
                    return
                # evacuate both PSUM ctx tiles to SBUF first (quick DVE
                # copies) so the next unit's PV matmuls get their PSUM
                # slots immediately; the normalize chain then runs from
                # SBUF off the PE critical path
                evac = []
                for hh in range(2):
                    ctmp = norm_pool.tile([D, TB], f32, tag="ctmp", name="ctmp")
                    nc.vector.tensor_copy(ctmp, ctx_ps[hh][0:D, :])
                    dcp = norm_pool.tile([1, TB], f32, tag="dcp", name="dcp")
                    nc.vector.tensor_copy(dcp, ctx_ps[hh][D : D + 1, :])
                    evac.append((ctmp, dcp))
                for hh in range(2):
                    ctmp, dcp = evac[hh]
                    rbc = norm_pool.tile([D, TB], f32, tag="rbc", name="rbc")
                    nc.gpsimd.partition_broadcast(rbc, dcp)
                    rec = norm_pool.tile([D, TB], f32, tag="rec", name="rec")
                    nc.vector.reciprocal_approx_fast(out=rec, in_=rbc)
                    dst = qT[p][hh * 64 : hh * 64 + 64, t0 : t0 + TB]
                    nc.vector.tensor_mul(dst, ctmp, rec)
                    if with_bv:
                        nc.vector.tensor_scalar_add(
                            dst, dst, bv_sb[64 * hh : 64 * hh + 64, p : p + 1]
                        )

            def emit_outproj_nh(tt, nh, on_scalar=False):
                ps_t = ps.tile([128, 512], f32, tag="proj", name="op_ps")
                for kk in range(2):
                    nc.tensor.matmul(
                        ps_t,
                        qT[kk][:, tt * 128 : (tt + 1) * 128],
                        wo_sb[:, kk, nh * 512 : (nh + 1) * 512],
                        start=(kk == 0),
                        stop=(kk == 1),
                    )
                o_sb = outs_pool.tile([128, 512], f32r, tag="osb", name="o_sb")
                # in the tail the ACT engine is idle; use it for the evac
                # so the PE<->DVE ping-pong disappears
                if on_scalar:
                    nc.scalar.copy(o_sb, ps_t)
                else:
                    nc.vector.tensor_copy(o_sb, ps_t)
                nc.sync.dma_start(
                    out=OUT[
                        tt * 128 : (tt + 1) * 128, nh * 512 : (nh + 1) * 512
                    ],
                    in_=o_sb,
                )

            def qproj_m_thunks(tb, m, cell):
                # 4 (m=0, incl. the xq prefetch) or 3 (m=1) small thunks;
                # chunks chain into one psum accumulation group
                def start(tb=tb):
                    cell["xq"] = emit_xq(tb)

                def chunk(j0, j1, fin, tb=tb, m=m):
                    if j0 == 0:
                        cell[m] = ps.tile(
                            [128, 512], f32, tag="proj", name="qp_ps"
                        )
                    ps_t = cell[m]
                    for j in range(j0, j1):
                        nc.tensor.matmul(
                            ps_t,
                            wq_sb[:, 2 * j : 2 * j + 2, m * 128 : (m + 1) * 128],
                            cell["xq"][:, 2 * j : 2 * j + 2, :],
                            start=(j == 0),
                            stop=(j == 3),
                            perf_mode=DR,
                        )
                    if fin:
                        nc.vector.tensor_scalar_add(
                            qT[m][:, tb * TB : (tb + 1) * TB],
                            ps_t,
                            bqk_sb[:, m : m + 1],
                        )

                out = [] if m else [start]
                out += [
                    lambda: chunk(0, 2, False),
                    lambda: chunk(2, 4, True),
                ]
                return out

            def outproj_thunks(tb, half, on_scalar=False):
                out = []
                for tt in range(4 * tb + 2 * half, 4 * tb + 2 * half + 2):
                    for nh in range(2):
                        out.append(
                            lambda tt=tt, nh=nh: emit_outproj_nh(
                                tt, nh, on_scalar
                            )
                        )
                return out

            # ---- emission schedule ----
            # prologue: kproj/qproj m0, prefill unit (0,0) scores so the
            # ACT engine starts exp-ing early, then the m1 halves + vproj
            # q-side first: the fp8 q tensors are 2.7x smaller than the
            # bf16 k-side, so qproj runs while wk/xk0 are still streaming
            nc.sync.dma_start(out=wq_sb, in_=WQ)
            xq0 = emit_xq(0)
            xk_ts = [emit_xk(0)]
            nc.sync.dma_start(out=wk_sb, in_=WK)
            xk_ts += [emit_xk(kb_i) for kb_i in range(1, len(kbs))]
            emit_qproj_m(0, 0, xq0)
            emit_qproj_m(0, 1, xq0)
            emit_kproj_m(0, 0, xk_ts[0])
            n_pre = min(4, nkt)
            prs0 = [emit_scores(0, 0, kt) for kt in range(n_pre)]
            for kb_i in range(1, len(kbs)):
                emit_kproj_m(kb_i, 0, xk_ts[kb_i])
            # remaining prefill scores run at ACT pace (the 2-buf sc
            # rotation waits on exp); weave the m1-half projections in
            # between so the PE keeps busy
            fill = [
                (lambda kb_i=kb_i: emit_kproj_m(kb_i, 1, xk_ts[kb_i]))
                for kb_i in range(len(kbs))
            ]
            fi = 0
            for kt in range(n_pre, nkt):
                prs0.append(emit_scores(0, 0, kt))
                if fi < len(fill):
                    fill[fi]()
                    fi += 1
            while fi < len(fill):
                fill[fi]()
                fi += 1
            n_vpre = min(4, nkt)
            for kt in range(n_vpre):
                emit_vproj_kt(kt)

            # unit-lag pipeline: while unit (p,tb) emits scores+exp, the
            # previous unit's PV matmuls drain against its ready probs.
            # vproj kt>=4 rides as unit-1 thunks: thunk[i] runs at step i,
            # and pv(0,0,kt) (needing v_sb[kt]) runs at step kt > i.
            units = [(p, tb) for tb in range(NTB) for p in range(2)]
            qcells = {tb: {} for tb in range(1, NTB)}
            vp_thunks = [
                (lambda kt=kt: emit_vproj_kt(kt)) for kt in range(n_vpre, nkt)
            ]
            thunk_map = {
                1: (lambda qp: [
                    th
                    for i in range(max(len(vp_thunks), len(qp)))
                    for th in (
                        vp_thunks[i : i + 1] + qp[i : i + 1]
                    )
                ])(qproj_m_thunks(1, 0, qcells[1])),
                2: qproj_m_thunks(1, 1, qcells[1]),
                3: qproj_m_thunks(2, 0, qcells[2]) + outproj_thunks(0, 0),
                4: qproj_m_thunks(2, 1, qcells[2]) + outproj_thunks(0, 1),
                5: qproj_m_thunks(3, 0, qcells[3]) + outproj_thunks(1, 0),
                6: qproj_m_thunks(3, 1, qcells[3]) + outproj_thunks(1, 1),
                7: outproj_thunks(2, 0) + outproj_thunks(2, 1),
            }
            prev_prs = prs0
            for ui in range(1, len(units)):
                p, tb = units[ui]
                pp, ptb = units[ui - 1]
                ctx_prev = [
                    ps.tile([D + 1, TB], f32, tag="ctx", name=f"ctx{pp}_{ptb}_{i}")
                    for i in range(2)
                ]
                thunks = thunk_map.get(ui, [])
                ti = 0
                cur_prs = []
                for kt in range(nkt):
                    cur_prs.append(emit_scores(p, tb, kt))
                    emit_pv(pp, ptb, kt, prev_prs[kt], ctx_prev)
                    if ti < len(thunks):
                        thunks[ti]()
                        ti += 1
                while ti < len(thunks):
                    thunks[ti]()
                    ti += 1
                emit_normalize(pp, ptb, ctx_prev)
                prev_prs = cur_prs
            # drain the last unit (1, NTB-1)
            ctx_last = [
                ps.tile([D + 1, TB], f32, tag="ctx", name=f"ctx_last_{i}")
                for i in range(2)
            ]
            for kt in range(nkt):
                emit_pv(1, NTB - 1, kt, prev_prs[kt], ctx_last)
            emit_normalize(1, NTB - 1, ctx_last, direct=True)
            for half in range(2):
                for th in outproj_thunks(3, half, on_scalar=True):
                    th()

    nc.compile()
    return nc


def kernel(
    query, key, value, Wq, bq, Wk, bk, Wv, bv, Wo, bo, query_mask, key_mask
):
    global LAST_EXEC_NS, LAST_TRACE_DIR
    from concourse.bass_utils import run_bass_kernel_spmd

    query = np.asarray(query, dtype=np.float32)
    key = np.asarray(key, dtype=np.float32)
    value = np.asarray(value, dtype=np.float32)
    Wq = np.asarray(Wq, dtype=np.float32)
    Wk = np.asarray(Wk, dtype=np.float32)
    Wv = np.asarray(Wv, dtype=np.float32)
    Wo = np.asarray(Wo, dtype=np.float32)
    bq = np.asarray(bq, dtype=np.float32)
    bk = np.asarray(bk, dtype=np.float32)
    bv = np.asarray(bv, dtype=np.float32)
    bo = np.asarray(bo, dtype=np.float32)
    qm = np.asarray(query_mask)
    km = np.asarray(key_mask)

    # host-side key compaction (query_mask masks the KEY axis, globally
    # per batch)
    keep = [np.flatnonzero(qm[b] != 0) for b in range(B)]
    nkeep = max((len(k) for k in keep), default=0)
    nkt = max(1, math.ceil(nkeep / 128))
    nkeys = nkt * 128

    with_bv = bool(np.any(bv))
    ck = (nkt, with_bv, BF16)
    if ck not in _CACHE:
        _CACHE[ck] = _build(nkt, with_bv, BF16)
    nc = _CACHE[ck]

    wdt = ml_dtypes.bfloat16 if BF16 else np.float32
    f8dt = ml_dtypes.float8_e4m3fn
    SX, SW = 16.0, 256.0  # fp8 pre-scales for the q/k path (2^4, 2^8)

    def arr_kmajor(a, ktiles, dt=None, s=1.0):  # [dim, n] -> [128, ktiles, n]
        return np.ascontiguousarray(
            (a * s if s != 1.0 else a)
            .reshape(ktiles, 128, a.shape[1])
            .transpose(1, 0, 2)
        ).astype(dt or wdt)

    in_maps = []
    for c in range(N_CORES):
        b, hg = c // HG, c % HG
        hs = hg * GH
        idx = keep[b]
        # compacted + padded key/value (transposed)
        xk = np.zeros((KDIM, nkeys), np.float32)
        xk[:, : len(idx)] = key[b].T[:, idx]
        xv = np.zeros((VDIM, nkeys), np.float32)
        xv[:, : len(idx)] = value[b].T[:, idx]
        mbias = np.full((nkeys,), NEG, np.float32)
        mbias[: len(idx)] = 0.0
        # qT/kT live on-device scaled by SX*SW = 2^12
        bqk = np.empty((128, 4), np.float32)
        bqk[:, 0] = bq[hs : hs + 128] * (SX * SW)
        bqk[:, 1] = bq[hs + 128 : hs + 256] * (SX * SW)
        bqk[:, 2] = bk[hs : hs + 128]
        bqk[:, 3] = bk[hs + 128 : hs + 256]
        m = {
            "xq": arr_kmajor(query[b].T, 8, f8dt, SX),
            "xk": arr_kmajor(xk, 6),
            "xv": arr_kmajor(xv, 6),
            "wq": arr_kmajor(Wq[:, hs : hs + GH], 8, f8dt, SW),
            "wk": arr_kmajor(Wk[:, hs : hs + GH], 6),
            "wv": arr_kmajor(Wv[:, hs : hs + GH], 6),
            "wo": arr_kmajor(Wo[hs : hs + GH, :], 2),
            "mbias": np.ascontiguousarray(mbias.reshape(nkt, 128).T),
            "bqk": bqk,
        }
        if with_bv:
            bvt = np.empty((128, 2), np.float32)
            bvt[:, 0] = bv[hs : hs + 128]
            bvt[:, 1] = bv[hs + 128 : hs + 256]
            m["bv"] = bvt
        in_maps.append(m)

    kwargs = {}
    if PROFILE:
        import tempfile

        LAST_TRACE_DIR = tempfile.mkdtemp(prefix="bass_trace_")
        kwargs = {"trace": True, "tmpdir": LAST_TRACE_DIR}
    res = run_bass_kernel_spmd(nc, in_maps, list(range(N_CORES)), **kwargs)
    LAST_EXEC_NS = res.exec_time_ns

    out = np.zeros((B, LQ, QDIM), np.float32)
    for c in range(N_CORES):
        out[c // HG] += res.results[c]["outp"].astype(np.float32)
    out += bo[None, None, :]
    for b in range(B):
        if len(keep[b]) == 0:
            # all keys masked: reference softmax is NaN everywhere
            out[b] = np.nan
    # key_mask masks the QUERY axis in the reference; a zero row makes the
    # whole softmax row -inf -> NaN output for that query position.
    for b in range(B):
        zq = np.flatnonzero(km[b] == 0)
        if len(zq):
            out[b, zq, :] = np.nan
    return out
